# revision 9
# baseline (speedup 1.0000x reference)
"""Trainium2 Bass kernel for a 6-layer GAT GNN (nn_GAT_GNN_35579509080109).

Strategy (8 NeuronCores, node partition):
  - Nodes are degree-balanced into 160 blocks of 128 slots (125 real nodes
    each); each device owns 20 blocks (2560 padded node slots).
  - Per layer, each device computes hw = h @ [W | W a_src | W a_dst] (+ folded
    bias) for its own nodes, packs a 768B row table
    [hw(256) bf16 | 1.0 | pad | e_src f32 | e_dst f32 | pad..384], and
    AllGathers the table.
  - Edges are partitioned by destination owner, sorted into dst blocks, and
    processed in chunks of 128 edges: hw[src]+e_src via a batched dma_gather
    (768B rows, by global src row), e_dst via a 256B aux-section dma_gather
    from the local table (by local dst row).
  - Per-edge softmax numerators ee = exp(leaky_relu(e_src+e_dst)) computed as
    max(exp(x), exp(0.2 x)) on the scalar engine in [128, GC] batches.
  - Scatter-add + denominators on the tensor engine: one-hot(dst slot) * ee
    (lhsT, bf16) x [hw | 1] (rhs) accumulates [128 dst, 257] in PSUM per
    block; out = psum[:, :256] / psum[:, 256].
  - Layer bias folds into the next layer's matmul (extra K=1 row); the final
    concat(h, h) @ W3 collapses to relu(h) @ (W3_top + W3_bot).
"""
import os
import sys
import numpy as np

for _p in ("/opt/trn_rl_repo", "/root/.axon_site/_ro/trn_rl_repo"):
    if os.path.isdir(_p) and _p not in sys.path:
        sys.path.append(_p)

# ---------------- problem constants ----------------
N = 20000
E = 320000
D = 256
NEG = 0.2
NDEV = 8

GC = 8    # chunks per gather group (1024 edges / dma_gather call; HW limit ~1024 idxs)
RW = 384  # table row width in bf16 (768 bytes)


class Cfg:
    def __init__(self, n, e, bpd):
        self.n, self.e, self.bpd = n, e, bpd
        self.npd = bpd * 128
        self.nblk = NDEV * bpd

FULL = Cfg(N, E, 20)


def _wrap16(flat):
    """dma_gather index layout: idx i at [i%16, i//16], replicated to 128 rows."""
    ni = flat.shape[0]
    w = np.ascontiguousarray(flat.reshape(ni // 16, 16).T).astype(np.int16)
    return np.tile(w, (8, 1))


# ---------------- host preprocessing ----------------
def prep(inputs, cfg):
    x = np.ascontiguousarray(np.asarray(inputs["x"], np.float32))
    ei = np.asarray(inputs["edge_index"]).astype(np.int64)
    W1 = np.asarray(inputs["W1"], np.float32)
    W2 = np.asarray(inputs["W2"], np.float32)
    Ws = np.asarray(inputs["Ws"], np.float32)
    a_src = np.asarray(inputs["a_src"], np.float32)
    a_dst = np.asarray(inputs["a_dst"], np.float32)
    bias = np.asarray(inputs["bias"], np.float32)
    W3 = np.asarray(inputs["W3"], np.float32)
    src, dst = ei[0], ei[1]
    n, bpd, npd, nblk = cfg.n, cfg.bpd, cfg.npd, cfg.nblk

    # --- degree-balanced node -> (dev, blk, slot) assignment (snake) ---
    deg = np.bincount(dst, minlength=n)
    order = np.argsort(-deg, kind="stable")
    r = np.arange(n)
    stripe = r // nblk
    posin = r % nblk
    blk_glob = np.where(stripe % 2 == 0, posin, nblk - 1 - posin)
    slot = stripe
    assert slot.max() < 128
    pos = np.empty(n, np.int64)
    pos[order] = (blk_glob // bpd) * npd + (blk_glob % bpd) * 128 + slot

    # --- edge grouping by dst block ---
    dstp = pos[dst]
    bid = dstp // npd * bpd + (dstp % npd) // 128  # global block id
    sidx = np.argsort(bid, kind="stable")
    counts = np.bincount(bid, minlength=nblk)
    cpb = int(np.ceil(counts.max() / 128))
    nchunk_raw = bpd * cpb
    nchunk = ((nchunk_raw + GC - 1) // GC) * GC
    starts = np.zeros(nblk + 1, np.int64)
    starts[1:] = np.cumsum(counts)
    rank = np.arange(cfg.e) - starts[bid[sidx]]

    sdev = (dstp // npd)[sidx]
    sblk = ((dstp % npd) // 128)[sidx]
    sslot = (dstp % 128)[sidx]
    ssrc = pos[src][sidx]
    kk = sblk * cpb + rank // 128
    pp = rank % 128

    SRC = np.zeros((NDEV, 128, nchunk), np.int32)       # global table row of src
    SLOT = np.full((NDEV, 128, nchunk), 255.0, np.float32)
    DSTL = np.zeros((NDEV, 128, nchunk), np.int32)      # local table row of dst
    SRC[sdev, pp, kk] = ssrc
    SLOT[sdev, pp, kk] = sslot
    DSTL[sdev, pp, kk] = (sblk * 128 + sslot).astype(np.int32)

    # wrapped int16 index arrays for dma_gather, per group of GC chunks
    ng = nchunk // GC
    wcols = GC * 128 // 16
    srcw = np.zeros((NDEV, 128, wcols * ng), np.int16)
    dstw = np.zeros((NDEV, 128, wcols * ng), np.int16)
    for dv in range(NDEV):
        for g in range(ng):
            # edge i in group = c*128 + p, c in [0,GC)
            flat_s = SRC[dv][:, g * GC:(g + 1) * GC].T.reshape(-1)  # [GC*128] c-major
            flat_d = DSTL[dv][:, g * GC:(g + 1) * GC].T.reshape(-1)
            srcw[dv][:, g * wcols:(g + 1) * wcols] = _wrap16(flat_s)
            dstw[dv][:, g * wcols:(g + 1) * wcols] = _wrap16(flat_d)

    # --- x permuted / padded / transposed ---
    xp = np.zeros((NDEV, npd, D), np.float32)
    xp[pos // npd, pos % npd] = x
    xpT = np.ascontiguousarray(xp.transpose(0, 2, 1))

    # --- weights ---
    W12 = np.ascontiguousarray(W1 @ W2)
    wfull = np.zeros((6, 257, 258), np.float32)
    for l in range(6):
        wext = np.concatenate(
            [Ws[l], (Ws[l] @ a_src[l])[:, None], (Ws[l] @ a_dst[l])[:, None]], axis=1
        )
        wfull[l, :256] = wext
        if l >= 1:
            wfull[l, 256] = bias[l - 1] @ wext
    W3s = np.ascontiguousarray(W3[:256] + W3[256:])
    bias6 = np.tile(bias[5][None, :], (128, 1)).astype(np.float32)
    iotaf = np.tile(np.arange(128, dtype=np.float32)[None, :], (128, 1))

    in_maps = []
    for dv in range(NDEV):
        in_maps.append(
            {
                "xT": np.ascontiguousarray(xpT[dv]),
                "srcw": np.ascontiguousarray(srcw[dv]),
                "dstw": np.ascontiguousarray(dstw[dv]),
                "slotf": np.ascontiguousarray(SLOT[dv]),
                "w12": W12,
                "wfull": wfull,
                "w3s": W3s,
                "bias6": bias6,
                "iotaf": iotaf,
            }
        )
    return in_maps, pos, cpb, nchunk


# ---------------- bass program ----------------
def build(cfg, cpb, nchunk):
    import concourse.bass as bass
    import concourse.bacc as bacc
    import concourse.tile as tile
    from concourse import mybir
    from concourse.masks import make_identity

    f32 = mybir.dt.float32
    bf16 = mybir.dt.bfloat16
    i16 = mybir.dt.int16
    AF = mybir.ActivationFunctionType
    OP = mybir.AluOpType
    npd, bpd = cfg.npd, cfg.bpd
    ng = nchunk // GC

    nc = bacc.Bacc(
        "TRN2",
        target_bir_lowering=False,
        debug=False,
        enable_asserts=False,
        num_devices=NDEV,
    )
    xT = nc.dram_tensor("xT", [256, npd], f32, kind="ExternalInput").ap()
    wcols = GC * 128 // 16
    srcw = nc.dram_tensor("srcw", [128, wcols * ng], i16, kind="ExternalInput").ap()
    dstw = nc.dram_tensor("dstw", [128, wcols * ng], i16, kind="ExternalInput").ap()
    slotf = nc.dram_tensor("slotf", [128, nchunk], f32, kind="ExternalInput").ap()
    w12 = nc.dram_tensor("w12", [256, 256], f32, kind="ExternalInput").ap()
    wfull = nc.dram_tensor("wfull", [6, 257, 258], f32, kind="ExternalInput").ap()
    w3s = nc.dram_tensor("w3s", [256, 256], f32, kind="ExternalInput").ap()
    bias6 = nc.dram_tensor("bias6", [128, 256], f32, kind="ExternalInput").ap()
    iotaf = nc.dram_tensor("iotaf", [128, 128], f32, kind="ExternalInput").ap()
    out = nc.dram_tensor("out", [npd, 256], f32, kind="ExternalOutput").ap()

    with tile.TileContext(nc) as tc:
        with (
            tc.tile_pool(name="cp", bufs=1) as cp,
            tc.tile_pool(name="sb", bufs=2) as sb,
            tc.tile_pool(name="gp", bufs=3) as gp,
            tc.tile_pool(name="psA", bufs=2, space="PSUM") as psA,
            tc.tile_pool(name="psB", bufs=2, space="PSUM") as psB,
            tc.tile_pool(name="dp", bufs=1, space="DRAM") as dp,
        ):
            # ---- constants ----
            ident = cp.tile([128, 128], f32)
            make_identity(nc, ident[:])
            iota_f = cp.tile([128, 128], f32)
            nc.sync.dma_start(iota_f[:], iotaf)
            iota_b = cp.tile([128, 128], bf16)
            nc.vector.tensor_copy(iota_b[:], iota_f[:])
            ones1 = cp.tile([1, 128], f32)
            nc.gpsimd.memset(ones1[:], 1.0)
            b6sb = cp.tile([128, 256], f32)
            nc.sync.dma_start(b6sb[:], bias6)
            w12sb = cp.tile([128, 2, 256], f32)
            nc.sync.dma_start(w12sb[:], w12.rearrange("(a p) m -> p a m", p=128))
            wextsb = cp.tile([128, 6, 2, 258], f32)
            for l in range(6):
                nc.sync.dma_start(
                    wextsb[:, l],
                    wfull[l, 0:256, :].rearrange("(a p) c -> p a c", p=128),
                )
            bwsb = cp.tile([1, 6, 258], f32)
            nc.sync.dma_start(bwsb[:], wfull[:, 256:257, :].rearrange("l o c -> o l c"))
            w3ssb = cp.tile([128, 2, 256], f32)
            nc.sync.dma_start(w3ssb[:], w3s.rearrange("(a p) m -> p a m", p=128))
            x0 = cp.tile([128, 2, npd], f32)
            nc.sync.dma_start(x0[:], xT.rearrange("(a p) n -> p a n", p=128))

            # ---- DRAM comm buffers (per layer: Shared tensors allow one writer) ----
            tbl_owns = [dp.tile([npd, RW], bf16, name=f"tbl_own{i}") for i in range(6)]
            tbl_fulls = [
                dp.tile([NDEV * npd, RW], bf16, addr_space="Shared", name=f"tbl_full{i}")
                for i in range(6)
            ]

            # ---- h0T = (x @ W12).T ----
            hT = sb.tile([128, 2, npd], f32, tag="hT")
            h0step = min(512, npd)
            for mm in range(2):
                for n0 in range(0, npd, h0step):
                    ps0 = psB.tile([128, h0step], f32, tag="pshw")
                    for ki in range(2):
                        nc.tensor.matmul(
                            ps0[:],
                            lhsT=w12sb[:, ki, mm * 128 : (mm + 1) * 128],
                            rhs=x0[:, ki, n0 : n0 + h0step],
                            start=(ki == 0),
                            stop=(ki == 1),
                        )
                    nc.vector.tensor_copy(hT[:, mm, n0 : n0 + h0step], ps0[:])

            for l in range(6):
                tbl_own, tbl_full = tbl_owns[l], tbl_fulls[l]
                # ---- own-node hw + table build ----
                tbl_sb = sb.tile([128, bpd, RW], bf16, tag="tbl")
                tblf = tbl_sb[:].bitcast(f32)  # [128, bpd, RW//2]
                nc.gpsimd.memset(tbl_sb[:, :, 256:RW], 1.0)
                for b in range(bpd):
                    pshw = psB.tile([128, 258], f32, tag="pshw")
                    for ki in range(2):
                        nc.tensor.matmul(
                            pshw[:],
                            lhsT=hT[:, ki, b * 128 : (b + 1) * 128],
                            rhs=wextsb[:, l, ki, :],
                            start=(ki == 0),
                            stop=False,
                        )
                    nc.tensor.matmul(
                        pshw[:],
                        lhsT=ones1[:],
                        rhs=bwsb[:, l, :],
                        start=False,
                        stop=True,
                    )
                    nc.vector.tensor_copy(tbl_sb[:, b, 0:256], pshw[:, 0:256])
                    nc.vector.tensor_copy(tblf[:, b, 129:131], pshw[:, 256:258])
                nc.sync.dma_start(
                    tbl_own[:].rearrange("(b p) c -> p b c", p=128), tbl_sb[:]
                )
                nc.gpsimd.collective_compute(
                    "AllGather",
                    mybir.AluOpType.bypass,
                    replica_groups=[list(range(NDEV))],
                    ins=[tbl_own[:]],
                    outs=[tbl_full[:]],
                )

                # ---- gather / scatter ----
                hT_next = sb.tile([128, 2, npd], f32, tag="hT")
                slott = G = EE = None
                for b in range(bpd):
                    pss = psA.tile([128, 257], f32, tag="pss")
                    for cc in range(cpb):
                        k = b * cpb + cc
                        g, c = divmod(k, GC)
                        if c == 0:
                            srci = gp.tile([128, wcols], i16, tag="srci")
                            nc.sync.dma_start(srci[:], srcw[:, g * wcols:(g + 1) * wcols])
                            dsti = gp.tile([128, wcols], i16, tag="dsti")
                            nc.sync.dma_start(dsti[:], dstw[:, g * wcols:(g + 1) * wcols])
                            slott = gp.tile([128, GC], f32, tag="slott")
                            nc.sync.dma_start(slott[:], slotf[:, g * GC:(g + 1) * GC])
                            G = gp.tile([128, GC, RW], bf16, tag="G")
                            nc.gpsimd.dma_gather(
                                out_ap=G[:], in_ap=tbl_full[:], idxs_ap=srci[:],
                                num_idxs=GC * 128, num_idxs_reg=GC * 128,
                                elem_size=RW,
                            )
                            ED = gp.tile([128, GC, 128], bf16, tag="ED")
                            nc.gpsimd.dma_gather(
                                out_ap=ED[:], in_ap=tbl_own[:, 256:RW],
                                idxs_ap=dsti[:],
                                num_idxs=GC * 128, num_idxs_reg=GC * 128,
                                elem_size=128, elem_step=RW,
                            )
                            Gf = G[:].bitcast(f32)    # [128, GC, 192]
                            EDf = ED[:].bitcast(f32)  # [128, GC, 64]
                            X = gp.tile([128, GC], f32, tag="X")
                            nc.vector.tensor_tensor(
                                X[:], Gf[:, :, 129], EDf[:, :, 2], op=OP.add
                            )
                            E1 = gp.tile([128, GC], f32, tag="E1")
                            nc.scalar.activation(E1[:], X[:], AF.Exp)
                            E2 = gp.tile([128, GC], f32, tag="E2")
                            nc.scalar.activation(E2[:], X[:], AF.Exp, scale=NEG)
                            EE = gp.tile([128, GC], f32, tag="EE")
                            nc.vector.tensor_tensor(EE[:], E1[:], E2[:], op=OP.max)
                        mask = gp.tile([128, 128], bf16, tag="mask")
                        nc.vector.tensor_scalar(
                            out=mask[:],
                            in0=iota_b[:],
                            scalar1=slott[:, c : c + 1],
                            scalar2=None,
                            op0=OP.is_equal,
                        )
                        lt = gp.tile([128, 128], bf16, tag="lt")
                        nc.vector.tensor_scalar(
                            out=lt[:],
                            in0=mask[:],
                            scalar1=EE[:, c : c + 1],
                            scalar2=None,
                            op0=OP.mult,
                        )
                        nc.tensor.matmul(
                            pss[:],
                            lhsT=lt[:],
                            rhs=G[:, c, 0:257],
                            start=(cc == 0),
                            stop=(cc == cpb - 1),
                        )
                    # ---- block epilogue: normalize ----
                    den = sb.tile([128, 1], f32, tag="den")
                    nc.vector.tensor_scalar(
                        out=den[:], in0=pss[:, 256:257], scalar1=1e-30,
                        scalar2=None, op0=OP.add,
                    )
                    rec = sb.tile([128, 1], f32, tag="rec")
                    nc.vector.reciprocal(rec[:], den[:])
                    onrm = sb.tile([128, 256], f32, tag="onrm")
                    nc.scalar.activation(onrm[:], pss[:, 0:256], AF.Copy, scale=rec[:])
                    if l == 5:
                        ob = sb.tile([128, 256], f32, tag="ob")
                        nc.vector.tensor_tensor(ob[:], onrm[:], b6sb[:], op=OP.add)
                        orl = sb.tile([128, 256], f32, tag="orl")
                        nc.vector.tensor_scalar(
                            out=orl[:], in0=ob[:], scalar1=0.0, scalar2=None, op0=OP.max
                        )
                        srct_t = orl
                    else:
                        srct_t = onrm
                    for hh in range(2):
                        pst = psB.tile([128, 128], f32, tag="pst")
                        nc.tensor.transpose(
                            out=pst[:],
                            in_=srct_t[:, hh * 128 : (hh + 1) * 128],
                            identity=ident[:],
                        )
                        nc.vector.tensor_copy(
                            hT_next[:, hh, b * 128 : (b + 1) * 128], pst[:]
                        )
                hT = hT_next

            # ---- final: relu(h6+bias) @ (W3_top + W3_bot) ----
            for b in range(bpd):
                psf = psB.tile([128, 256], f32, tag="pshw")
                for ki in range(2):
                    nc.tensor.matmul(
                        psf[:],
                        lhsT=hT[:, ki, b * 128 : (b + 1) * 128],
                        rhs=w3ssb[:, ki, :],
                        start=(ki == 0),
                        stop=(ki == 1),
                    )
                oo = sb.tile([128, 256], f32, tag="oo")
                nc.vector.tensor_copy(oo[:], psf[:])
                nc.sync.dma_start(out[b * 128 : (b + 1) * 128, :], oo[:])

    nc.compile()
    return nc


# ---------------- entry point ----------------
def kernel(**inputs):
    cfg = FULL
    in_maps, pos, cpb, nchunk = prep(inputs, cfg)
    nc = build(cfg, cpb, nchunk)
    from concourse import bass_utils

    res = bass_utils.run_bass_kernel_spmd(nc, in_maps, core_ids=list(range(NDEV)))
    outs = [res.results[dv]["out"] for dv in range(NDEV)]
    full = np.zeros((cfg.n, 256), np.float32)
    full[:] = np.stack(outs).reshape(NDEV * cfg.npd, 256)[pos]
    return full


# revision 11
# speedup vs baseline: 1.0882x; 1.0882x over previous
"""Trainium2 Bass kernel for a 6-layer GAT GNN (nn_GAT_GNN_35579509080109).

Strategy (8 NeuronCores, node partition):
  - Nodes are degree-balanced into 160 blocks of 128 slots (125 real nodes
    each); each device owns 20 blocks (2560 padded node slots).
  - Per layer, each device computes hw = h @ [W | W a_src | W a_dst] (+ folded
    bias) for its own nodes, packs a 768B row table
    [hw(256) bf16 | 1.0 | pad | e_src f32 | e_dst f32 | pad..384], and
    AllGathers the table.
  - Edges are partitioned by destination owner, sorted into dst blocks, and
    processed in chunks of 128 edges: hw[src]+e_src via a batched dma_gather
    (768B rows, by global src row), e_dst via a 256B aux-section dma_gather
    from the local table (by local dst row).
  - Per-edge softmax numerators ee = exp(leaky_relu(e_src+e_dst)) computed as
    max(exp(x), exp(0.2 x)) on the scalar engine in [128, GC] batches.
  - Scatter-add + denominators on the tensor engine: one-hot(dst slot) * ee
    (lhsT, bf16) x [hw | 1] (rhs) accumulates [128 dst, 257] in PSUM per
    block; out = psum[:, :256] / psum[:, 256].
  - Layer bias folds into the next layer's matmul (extra K=1 row); the final
    concat(h, h) @ W3 collapses to relu(h) @ (W3_top + W3_bot).
"""
import os
import sys
import numpy as np

for _p in ("/opt/trn_rl_repo", "/root/.axon_site/_ro/trn_rl_repo"):
    if os.path.isdir(_p) and _p not in sys.path:
        sys.path.append(_p)

# ---------------- problem constants ----------------
N = 20000
E = 320000
D = 256
NEG = 0.2
NDEV = 8

GC = 8    # chunks per gather group (1024 edges / dma_gather call; HW limit ~1024 idxs)
RW = 384  # table row width in bf16 (768 bytes)


class Cfg:
    def __init__(self, n, e, bpd):
        self.n, self.e, self.bpd = n, e, bpd
        self.npd = bpd * 128
        self.nblk = NDEV * bpd

FULL = Cfg(N, E, 20)


def _wrap16(flat):
    """dma_gather index layout: idx i at [i%16, i//16], replicated to 128 rows."""
    ni = flat.shape[0]
    w = np.ascontiguousarray(flat.reshape(ni // 16, 16).T).astype(np.int16)
    return np.tile(w, (8, 1))


# ---------------- host preprocessing ----------------
def prep(inputs, cfg):
    x = np.ascontiguousarray(np.asarray(inputs["x"], np.float32))
    ei = np.asarray(inputs["edge_index"]).astype(np.int64)
    W1 = np.asarray(inputs["W1"], np.float32)
    W2 = np.asarray(inputs["W2"], np.float32)
    Ws = np.asarray(inputs["Ws"], np.float32)
    a_src = np.asarray(inputs["a_src"], np.float32)
    a_dst = np.asarray(inputs["a_dst"], np.float32)
    bias = np.asarray(inputs["bias"], np.float32)
    W3 = np.asarray(inputs["W3"], np.float32)
    src, dst = ei[0], ei[1]
    n, bpd, npd, nblk = cfg.n, cfg.bpd, cfg.npd, cfg.nblk

    # --- degree-balanced node -> (dev, blk, slot) assignment (snake) ---
    deg = np.bincount(dst, minlength=n)
    order = np.argsort(-deg, kind="stable")
    r = np.arange(n)
    stripe = r // nblk
    posin = r % nblk
    blk_glob = np.where(stripe % 2 == 0, posin, nblk - 1 - posin)
    slot = stripe
    assert slot.max() < 128
    pos = np.empty(n, np.int64)
    pos[order] = (blk_glob // bpd) * npd + (blk_glob % bpd) * 128 + slot

    # --- edge grouping by dst block ---
    dstp = pos[dst]
    bid = dstp // npd * bpd + (dstp % npd) // 128  # global block id
    sidx = np.argsort(bid, kind="stable")
    counts = np.bincount(bid, minlength=nblk)
    cpb = int(np.ceil(counts.max() / 128))
    nchunk_raw = bpd * cpb
    nchunk = ((nchunk_raw + GC - 1) // GC) * GC
    starts = np.zeros(nblk + 1, np.int64)
    starts[1:] = np.cumsum(counts)
    rank = np.arange(cfg.e) - starts[bid[sidx]]

    sdev = (dstp // npd)[sidx]
    sblk = ((dstp % npd) // 128)[sidx]
    sslot = (dstp % 128)[sidx]
    ssrc = pos[src][sidx]
    kk = sblk * cpb + rank // 128
    pp = rank % 128

    SRC = np.zeros((NDEV, 128, nchunk), np.int32)       # global table row of src
    SLOT = np.full((NDEV, 128, nchunk), 255.0, np.float32)
    DSTL = np.zeros((NDEV, 128, nchunk), np.int32)      # local table row of dst
    SRC[sdev, pp, kk] = ssrc
    SLOT[sdev, pp, kk] = sslot
    DSTL[sdev, pp, kk] = (sblk * 128 + sslot).astype(np.int32)

    # wrapped int16 index arrays for dma_gather, per group of GC chunks
    ng = nchunk // GC
    wcols = GC * 128 // 16
    srcw = np.zeros((NDEV, 128, wcols * ng), np.int16)
    dstw = np.zeros((NDEV, 128, wcols * ng), np.int16)
    for dv in range(NDEV):
        for g in range(ng):
            # edge i in group = c*128 + p, c in [0,GC)
            flat_s = SRC[dv][:, g * GC:(g + 1) * GC].T.reshape(-1)  # [GC*128] c-major
            flat_d = DSTL[dv][:, g * GC:(g + 1) * GC].T.reshape(-1)
            srcw[dv][:, g * wcols:(g + 1) * wcols] = _wrap16(flat_s)
            dstw[dv][:, g * wcols:(g + 1) * wcols] = _wrap16(flat_d)

    # --- x permuted / padded / transposed ---
    xp = np.zeros((NDEV, npd, D), np.float32)
    xp[pos // npd, pos % npd] = x
    xpT = np.ascontiguousarray(xp.transpose(0, 2, 1))

    # --- weights ---
    W12 = np.ascontiguousarray(W1 @ W2)
    wfull = np.zeros((6, 257, 258), np.float32)
    for l in range(6):
        wext = np.concatenate(
            [Ws[l], (Ws[l] @ a_src[l])[:, None], (Ws[l] @ a_dst[l])[:, None]], axis=1
        )
        wfull[l, :256] = wext
        if l >= 1:
            wfull[l, 256] = bias[l - 1] @ wext
    W3s = np.ascontiguousarray(W3[:256] + W3[256:])
    bias6 = np.tile(bias[5][None, :], (128, 1)).astype(np.float32)
    iotaf = np.tile(np.arange(128, dtype=np.float32)[None, :], (128, 1))

    in_maps = []
    for dv in range(NDEV):
        in_maps.append(
            {
                "xT": np.ascontiguousarray(xpT[dv]),
                "srcw": np.ascontiguousarray(srcw[dv]),
                "dstw": np.ascontiguousarray(dstw[dv]),
                "slotf": np.ascontiguousarray(SLOT[dv]),
                "w12": W12,
                "wfull": wfull,
                "w3s": W3s,
                "bias6": bias6,
                "iotaf": iotaf,
            }
        )
    return in_maps, pos, cpb, nchunk


# ---------------- bass program ----------------
def build(cfg, cpb, nchunk):
    import concourse.bass as bass
    import concourse.bacc as bacc
    import concourse.tile as tile
    from concourse import mybir
    from concourse.masks import make_identity

    f32 = mybir.dt.float32
    bf16 = mybir.dt.bfloat16
    i16 = mybir.dt.int16
    AF = mybir.ActivationFunctionType
    OP = mybir.AluOpType
    npd, bpd = cfg.npd, cfg.bpd
    ng = nchunk // GC

    nc = bacc.Bacc(
        "TRN2",
        target_bir_lowering=False,
        debug=False,
        enable_asserts=False,
        num_devices=NDEV,
    )
    xT = nc.dram_tensor("xT", [256, npd], f32, kind="ExternalInput").ap()
    wcols = GC * 128 // 16
    srcw = nc.dram_tensor("srcw", [128, wcols * ng], i16, kind="ExternalInput").ap()
    dstw = nc.dram_tensor("dstw", [128, wcols * ng], i16, kind="ExternalInput").ap()
    slotf = nc.dram_tensor("slotf", [128, nchunk], f32, kind="ExternalInput").ap()
    w12 = nc.dram_tensor("w12", [256, 256], f32, kind="ExternalInput").ap()
    wfull = nc.dram_tensor("wfull", [6, 257, 258], f32, kind="ExternalInput").ap()
    w3s = nc.dram_tensor("w3s", [256, 256], f32, kind="ExternalInput").ap()
    bias6 = nc.dram_tensor("bias6", [128, 256], f32, kind="ExternalInput").ap()
    iotaf = nc.dram_tensor("iotaf", [128, 128], f32, kind="ExternalInput").ap()
    out = nc.dram_tensor("out", [npd, 256], f32, kind="ExternalOutput").ap()

    with tile.TileContext(nc) as tc:
        with (
            tc.tile_pool(name="cp", bufs=1) as cp,
            tc.tile_pool(name="sb", bufs=2) as sb,
            tc.tile_pool(name="gp", bufs=4) as gp,
            tc.tile_pool(name="psA", bufs=3, space="PSUM") as psA,
            tc.tile_pool(name="psB", bufs=2, space="PSUM") as psB,
            tc.tile_pool(name="dp", bufs=1, space="DRAM") as dp,
        ):
            # ---- constants ----
            ident = cp.tile([128, 128], f32)
            make_identity(nc, ident[:])
            iota_f = cp.tile([128, 128], f32)
            nc.sync.dma_start(iota_f[:], iotaf)
            iota_b = cp.tile([128, 128], bf16)
            nc.vector.tensor_copy(iota_b[:], iota_f[:])
            ones1 = cp.tile([1, 128], f32)
            nc.gpsimd.memset(ones1[:], 1.0)
            b6sb = cp.tile([128, 256], f32)
            nc.sync.dma_start(b6sb[:], bias6)
            w12sb = cp.tile([128, 2, 256], f32)
            nc.sync.dma_start(w12sb[:], w12.rearrange("(a p) m -> p a m", p=128))
            wextsb = cp.tile([128, 6, 2, 258], f32)
            for l in range(6):
                nc.sync.dma_start(
                    wextsb[:, l],
                    wfull[l, 0:256, :].rearrange("(a p) c -> p a c", p=128),
                )
            bwsb = cp.tile([1, 6, 258], f32)
            nc.sync.dma_start(bwsb[:], wfull[:, 256:257, :].rearrange("l o c -> o l c"))
            w3ssb = cp.tile([128, 2, 256], f32)
            nc.sync.dma_start(w3ssb[:], w3s.rearrange("(a p) m -> p a m", p=128))
            x0 = cp.tile([128, 2, npd], f32)
            nc.sync.dma_start(x0[:], xT.rearrange("(a p) n -> p a n", p=128))

            # ---- DRAM comm buffers (per layer: Shared tensors allow one writer) ----
            tbl_owns = [dp.tile([npd, RW], bf16, name=f"tbl_own{i}") for i in range(6)]
            tbl_fulls = [
                dp.tile([NDEV * npd, RW], bf16, addr_space="Shared", name=f"tbl_full{i}")
                for i in range(6)
            ]

            # ---- h0T = (x @ W12).T ----
            hT = sb.tile([128, 2, npd], f32, tag="hT")
            h0step = min(512, npd)
            for mm in range(2):
                for n0 in range(0, npd, h0step):
                    ps0 = psB.tile([128, h0step], f32, tag="pshw")
                    for ki in range(2):
                        nc.tensor.matmul(
                            ps0[:],
                            lhsT=w12sb[:, ki, mm * 128 : (mm + 1) * 128],
                            rhs=x0[:, ki, n0 : n0 + h0step],
                            start=(ki == 0),
                            stop=(ki == 1),
                        )
                    nc.vector.tensor_copy(hT[:, mm, n0 : n0 + h0step], ps0[:])

            for l in range(6):
                tbl_own, tbl_full = tbl_owns[l], tbl_fulls[l]
                # ---- own-node hw + table build ----
                tbl_sb = sb.tile([128, bpd, RW], bf16, tag="tbl")
                tblf = tbl_sb[:].bitcast(f32)  # [128, bpd, RW//2]
                nc.gpsimd.memset(tbl_sb[:, :, 256:RW], 1.0)
                for b in range(bpd):
                    pshw = psB.tile([128, 258], f32, tag="pshw")
                    for ki in range(2):
                        nc.tensor.matmul(
                            pshw[:],
                            lhsT=hT[:, ki, b * 128 : (b + 1) * 128],
                            rhs=wextsb[:, l, ki, :],
                            start=(ki == 0),
                            stop=False,
                        )
                    nc.tensor.matmul(
                        pshw[:],
                        lhsT=ones1[:],
                        rhs=bwsb[:, l, :],
                        start=False,
                        stop=True,
                    )
                    nc.vector.tensor_copy(tbl_sb[:, b, 0:256], pshw[:, 0:256])
                    nc.vector.tensor_copy(tblf[:, b, 129:131], pshw[:, 256:258])
                nc.sync.dma_start(
                    tbl_own[:].rearrange("(b p) c -> p b c", p=128), tbl_sb[:]
                )
                nc.gpsimd.collective_compute(
                    "AllGather",
                    mybir.AluOpType.bypass,
                    replica_groups=[list(range(NDEV))],
                    ins=[tbl_own[:]],
                    outs=[tbl_full[:]],
                )

                # ---- gather / scatter ----
                hT_next = sb.tile([128, 2, npd], f32, tag="hT")
                slott = G = EE = None
                for b in range(bpd):
                    pss = psA.tile([128, 257], f32, tag="pss")
                    for cc in range(cpb):
                        k = b * cpb + cc
                        g, c = divmod(k, GC)
                        if c == 0:
                            srci = gp.tile([128, wcols], i16, tag="srci")
                            nc.sync.dma_start(srci[:], srcw[:, g * wcols:(g + 1) * wcols])
                            dsti = gp.tile([128, wcols], i16, tag="dsti")
                            nc.sync.dma_start(dsti[:], dstw[:, g * wcols:(g + 1) * wcols])
                            slott = gp.tile([128, GC], f32, tag="slott")
                            nc.sync.dma_start(slott[:], slotf[:, g * GC:(g + 1) * GC])
                            G = gp.tile([128, GC, RW], bf16, tag="G")
                            nc.gpsimd.dma_gather(
                                out_ap=G[:], in_ap=tbl_full[:], idxs_ap=srci[:],
                                num_idxs=GC * 128, num_idxs_reg=GC * 128,
                                elem_size=RW,
                            )
                            ED = gp.tile([128, GC, 128], bf16, tag="ED")
                            nc.gpsimd.dma_gather(
                                out_ap=ED[:], in_ap=tbl_own[:, 256:RW],
                                idxs_ap=dsti[:],
                                num_idxs=GC * 128, num_idxs_reg=GC * 128,
                                elem_size=128, elem_step=RW,
                            )
                            Gf = G[:].bitcast(f32)    # [128, GC, 192]
                            EDf = ED[:].bitcast(f32)  # [128, GC, 64]
                            X = gp.tile([128, GC], f32, tag="X")
                            nc.vector.tensor_tensor(
                                X[:], Gf[:, :, 129], EDf[:, :, 2], op=OP.add
                            )
                            E1 = gp.tile([128, GC], f32, tag="E1")
                            nc.scalar.activation(E1[:], X[:], AF.Exp)
                            E2 = gp.tile([128, GC], f32, tag="E2")
                            nc.scalar.activation(E2[:], X[:], AF.Exp, scale=NEG)
                            EE = gp.tile([128, GC], f32, tag="EE")
                            nc.vector.tensor_tensor(EE[:], E1[:], E2[:], op=OP.max)
                        lt = gp.tile([128, 128], bf16, tag="lt")
                        nc.vector.tensor_scalar(
                            out=lt[:],
                            in0=iota_b[:],
                            scalar1=slott[:, c : c + 1],
                            scalar2=EE[:, c : c + 1],
                            op0=OP.is_equal,
                            op1=OP.mult,
                        )
                        nc.tensor.matmul(
                            pss[:],
                            lhsT=lt[:],
                            rhs=G[:, c, 0:257],
                            start=(cc == 0),
                            stop=(cc == cpb - 1),
                        )
                    # ---- block epilogue: normalize ----
                    den = sb.tile([128, 1], f32, tag="den")
                    nc.vector.tensor_scalar(
                        out=den[:], in0=pss[:, 256:257], scalar1=1e-30,
                        scalar2=None, op0=OP.add,
                    )
                    rec = sb.tile([128, 1], f32, tag="rec")
                    nc.vector.reciprocal(rec[:], den[:])
                    onrm = sb.tile([128, 256], f32, tag="onrm")
                    nc.scalar.activation(onrm[:], pss[:, 0:256], AF.Copy, scale=rec[:])
                    if l == 5:
                        ob = sb.tile([128, 256], f32, tag="ob")
                        nc.vector.tensor_tensor(ob[:], onrm[:], b6sb[:], op=OP.add)
                        orl = sb.tile([128, 256], f32, tag="orl")
                        nc.vector.tensor_scalar(
                            out=orl[:], in0=ob[:], scalar1=0.0, scalar2=None, op0=OP.max
                        )
                        srct_t = orl
                    else:
                        srct_t = onrm
                    for hh in range(2):
                        pst = psB.tile([128, 128], f32, tag="pst")
                        nc.tensor.transpose(
                            out=pst[:],
                            in_=srct_t[:, hh * 128 : (hh + 1) * 128],
                            identity=ident[:],
                        )
                        nc.vector.tensor_copy(
                            hT_next[:, hh, b * 128 : (b + 1) * 128], pst[:]
                        )
                hT = hT_next

            # ---- final: relu(h6+bias) @ (W3_top + W3_bot) ----
            for b in range(bpd):
                psf = psB.tile([128, 256], f32, tag="pshw")
                for ki in range(2):
                    nc.tensor.matmul(
                        psf[:],
                        lhsT=hT[:, ki, b * 128 : (b + 1) * 128],
                        rhs=w3ssb[:, ki, :],
                        start=(ki == 0),
                        stop=(ki == 1),
                    )
                oo = sb.tile([128, 256], f32, tag="oo")
                nc.vector.tensor_copy(oo[:], psf[:])
                nc.sync.dma_start(out[b * 128 : (b + 1) * 128, :], oo[:])

    nc.compile()
    return nc


# ---------------- entry point ----------------
def kernel(**inputs):
    cfg = FULL
    in_maps, pos, cpb, nchunk = prep(inputs, cfg)
    nc = build(cfg, cpb, nchunk)
    from concourse import bass_utils

    res = bass_utils.run_bass_kernel_spmd(nc, in_maps, core_ids=list(range(NDEV)))
    outs = [res.results[dv]["out"] for dv in range(NDEV)]
    full = np.zeros((cfg.n, 256), np.float32)
    full[:] = np.stack(outs).reshape(NDEV * cfg.npd, 256)[pos]
    return full
